# revision 18
# baseline (speedup 1.0000x reference)
"""AdaptiveLocalPositionEmbedding Trainium2 kernel (8 NeuronCores, data parallel).

out[b,s,:] = x[b,s,:] + pos_emb[b,s,:] where pos_emb is
  control_emb[s] (s<4, before any start segment), sequence_emb[s-last] for the
  latest start token position last<=s (planted at pos>=4, rel<1003), else 0.

Per core (2 batch rows, 4096 tokens): device computes the segment scan
(cummax over start-token markers), per-token table indices, then gathers
2KB table rows with dma_gather (4x1024 rows over 4 SWDGE queues) and adds
into 1MiB-batched x tiles. The combined table (4 ctrl + 1003 seq + zero row)
is gathered in bf16 to halve HBM gather traffic. Host does dtype casts, the
==start_token_id compare (runtime scalar), layout packing, shard/unshard.

Token layout: gather slot i <-> token t=i (t = b*2048 + s). dma_gather wants
idx i at partition i%16, col i//16 (replicated x8 across partition groups) and
writes row i to dst[i%128, i//128] - exactly matching x tiles [128, c, 512].
The index pipeline therefore runs in a transposed space: markerT[mm, b*16+g]
for s = mm*16+g, scan over g (free dim), block-scan over mm via PE transpose,
final PE transpose emits G[g, b*128+mm] = idx int16.
"""

import os
import sys

import numpy as np

for _p in ("/opt/trn_rl_repo",):
    if _p not in sys.path:
        sys.path.insert(0, _p)

import ml_dtypes

from concourse import bacc, bass, library_config, mybir
from concourse.bass_utils import run_bass_kernel_spmd
from concourse.tile import TileContext

B, S, D = 16, 2048, 512
N_CORES = 8
B_SH = B // N_CORES            # 2 batch rows per core
TOK = B_SH * S                 # 4096 tokens per core
N_CTRL = 4
N_SEQ = 1003
ZERO_ROW = N_CTRL + N_SEQ      # 1007
TBL = ZERO_ROW + 1             # 1008 rows
NG = 4                         # gather calls (1024 rows each, one per queue)
GI = TOK // NG                 # 1024 idxs per gather
F32 = mybir.dt.float32
BF16 = mybir.dt.bfloat16
I16 = mybir.dt.int16

_CACHE = {}


def _ensure_ntff_hook():
    """The agent image's antenv package lacks axon_hooks, so NTFF tracing
    silently degrades. Synthesize the module and register the boot script's
    ctypes-based profile hook so trace=True yields exec_time_ns."""
    if "antenv.axon_hooks" in sys.modules:
        return
    try:
        import types

        import antenv
        from trn_agent_boot.trn_boot import _ntff_profile_via_ctypes

        mod = types.ModuleType("antenv.axon_hooks")
        mod._hook = None

        def set_axon_ntff_profile_hook(h):
            mod._hook = h

        def get_axon_ntff_profile_hook():
            return mod._hook

        mod.set_axon_ntff_profile_hook = set_axon_ntff_profile_hook
        mod.get_axon_ntff_profile_hook = get_axon_ntff_profile_hook
        sys.modules["antenv.axon_hooks"] = mod
        antenv.axon_hooks = mod
        mod._hook = _ntff_profile_via_ctypes("/opt/axon/libaxon_pjrt.so")
    except Exception as e:  # tracing degrades; run still works
        print(f"NTFF hook registration failed: {e}", file=sys.stderr)


def _build_bass():
    nc = bacc.Bacc("TRN2", num_swdge_queues=4)
    x_h = nc.dram_tensor("x", [TOK, D], F32, kind="ExternalInput")
    # consts [128, 192]: 0:128 id128, 128:144 svalp1T, 144:160 baseT,
    # 160:192 markerT (per-core start-token markers, transposed layout)
    cst_h = nc.dram_tensor("consts", [128, 192], F32, kind="ExternalInput")
    table_h = nc.dram_tensor("table", [TBL, D], BF16, kind="ExternalInput")
    out_h = nc.dram_tensor("out", [TOK, D], F32, kind="ExternalOutput")

    with TileContext(nc) as tc:
        with (
            tc.tile_pool(name="const", bufs=1) as cpool,
            tc.tile_pool(name="work", bufs=4) as wpool,
            tc.tile_pool(name="psum", bufs=1, space="PSUM") as ppool,
        ):
            # pull the gpsimd DMA-gather ucode in during startup, not at
            # first-gather time (the reload stalls the gather stream)
            nc.gpsimd.load_library(library_config.mlp)
            cst = cpool.tile([128, 192], F32)
            nc.gpsimd.dma_start(out=cst[:], in_=cst_h[:])
            id128 = cst[:, 0:128]
            id1 = cst[0:1, 0:1]
            svalp1T = cst[:, 128:144]
            baseT = cst[:, 144:160]

            # G[g, b*128+mm] = table index for token t=b*2048+mm*16+g, int16,
            # replicated x8 across 16-partition groups for the DGE cores.
            G = cpool.tile([128, 2 * 128], I16)

            for b in range(B_SH):
                mk = cst[:, 160 + 16 * b:160 + 16 * (b + 1)]   # [128,16]
                # inclusive cummax along g (within each 16-token column)
                sA = cpool.tile([128, 16], F32, tag=f"sA{b}")
                sB = cpool.tile([128, 16], F32, tag=f"sB{b}")
                nc.vector.tensor_copy(out=sA[:], in_=mk)
                cur, nxt = sA, sB
                for k in (1, 2, 4, 8):
                    nc.vector.tensor_copy(out=nxt[:, :k], in_=cur[:, :k])
                    nc.vector.tensor_tensor(out=nxt[:, k:], in0=cur[:, k:],
                                            in1=cur[:, :16 - k],
                                            op=mybir.AluOpType.max)
                    cur, nxt = nxt, cur
                # cross-column exclusive cummax over mm (128 columns)
                cmT_ps = ppool.tile([1, 128], F32, space="PSUM", tag=f"cm{b}")
                nc.tensor.matmul(out=cmT_ps[:], lhsT=cur[:, 15:16], rhs=id128,
                                 start=True, stop=True)
                ex = cpool.tile([1, 128], F32, tag=f"ex{b}")
                ex2 = cpool.tile([1, 128], F32, tag=f"ex2{b}")
                nc.vector.memset(ex[:], -1.0)
                nc.vector.tensor_copy(out=ex[:, 1:128], in_=cmT_ps[:, 0:127])
                curX, nxtX = ex, ex2
                for k in (1, 2, 4, 8, 16, 32, 64):
                    nc.vector.tensor_copy(out=nxtX[:, :k], in_=curX[:, :k])
                    nc.vector.tensor_tensor(out=nxtX[:, k:], in0=curX[:, k:],
                                            in1=curX[:, :128 - k],
                                            op=mybir.AluOpType.max)
                    curX, nxtX = nxtX, curX
                pref_ps = ppool.tile([128, 1], F32, space="PSUM", tag=f"pf{b}")
                nc.tensor.matmul(out=pref_ps[:], lhsT=curX[:], rhs=id1,
                                 start=True, stop=True)
                pref = cpool.tile([128, 1], F32, tag=f"pref{b}")
                nc.vector.tensor_copy(out=pref[:], in_=pref_ps[:])

                last = nxt  # reuse scan ping buffer
                nc.vector.tensor_tensor(out=last[:], in0=cur[:],
                                        in1=pref[:, 0:1].to_broadcast([128, 16]),
                                        op=mybir.AluOpType.max)
                ge0 = cpool.tile([128, 16], F32, tag=f"ge0{b}")
                nc.vector.tensor_scalar(out=ge0[:], in0=last[:], scalar1=0.0,
                                        scalar2=None, op0=mybir.AluOpType.is_ge)
                rel4 = cpool.tile([128, 16], F32, tag=f"rel4{b}")
                # rel+4 = (s+1) + 3 - last
                nc.vector.tensor_tensor(out=rel4[:], in0=svalp1T, in1=last[:],
                                        op=mybir.AluOpType.subtract)
                nc.vector.tensor_scalar_add(out=rel4[:], in0=rel4[:], scalar1=3.0)
                le = cpool.tile([128, 16], F32, tag=f"le{b}")
                nc.vector.tensor_scalar(out=le[:], in0=rel4[:], scalar1=1006.0,
                                        scalar2=None, op0=mybir.AluOpType.is_le)
                valid = cpool.tile([128, 16], F32, tag=f"va{b}")
                nc.vector.tensor_tensor(out=valid[:], in0=ge0[:], in1=le[:],
                                        op=mybir.AluOpType.mult)
                idxf = cpool.tile([128, 16], F32, tag=f"ix{b}")
                # idx = base + valid * (rel4 - base)
                nc.vector.tensor_tensor(out=idxf[:], in0=rel4[:], in1=baseT,
                                        op=mybir.AluOpType.subtract)
                nc.vector.tensor_tensor(out=idxf[:], in0=idxf[:], in1=valid[:],
                                        op=mybir.AluOpType.mult)
                nc.vector.tensor_tensor(out=idxf[:], in0=idxf[:], in1=baseT,
                                        op=mybir.AluOpType.add)

                gT_ps = ppool.tile([16, 128], F32, space="PSUM", tag=f"gt{b}")
                nc.tensor.matmul(out=gT_ps[:], lhsT=idxf[:], rhs=id128,
                                 start=True, stop=True)
                nc.vector.tensor_copy(out=G[0:16, 128 * b:128 * (b + 1)],
                                      in_=gT_ps[:])
                # replicate this half to the other 7 partition groups
                for r in range(1, 8):
                    nc.sync.dma_start(
                        out=G[16 * r:16 * (r + 1), 128 * b:128 * (b + 1)],
                        in_=G[0:16, 128 * b:128 * (b + 1)])

            # ---- main data path: 4 macro tiles of 1024 tokens ----
            for j in range(NG):
                xt = wpool.tile([128, GI // 128, D], F32, tag="xt")
                emb = wpool.tile([128, GI // 128, D], BF16, tag="emb")
                nc.sync.dma_start(
                    out=xt[:, :, :],
                    in_=x_h[GI * j:GI * (j + 1), :].rearrange(
                        "(c p) d -> p c d", c=GI // 128, p=128))
                nc.gpsimd.dma_gather(
                    emb[:, :, :], table_h[:],
                    G[:, (GI // 16) * j:(GI // 16) * (j + 1)],
                    GI, GI, D, queue_num=j)
                nc.vector.tensor_tensor(
                    out=xt[:, :, :], in0=xt[:, :, :], in1=emb[:, :, :],
                    op=mybir.AluOpType.add)
                nc.sync.dma_start(
                    out=out_h[GI * j:GI * (j + 1), :].rearrange(
                        "(c p) d -> p c d", c=GI // 128, p=128),
                    in_=xt[:, :, :])
    nc.compile()
    return nc


def _consts(mask):
    """mask: [2, 2048] float 0/1 start-token mask for this core's rows."""
    s = (np.arange(S, dtype=np.float32).reshape(128, 16, order="F"))
    # s[mm, g] = mm*16+g  (order F: index = mm + 128*?) -- build directly:
    mm = np.arange(128, dtype=np.float32)[:, None]
    g = np.arange(16, dtype=np.float32)[None, :]
    sv = mm * 16 + g                                        # [128,16] s value
    svalp1T = (sv + 1.0).astype(np.float32)
    baseT = np.where(sv < N_CTRL, sv, float(ZERO_ROW)).astype(np.float32)
    id128 = np.eye(128, dtype=np.float32)
    markerT = np.empty((128, 2 * 16), dtype=np.float32)
    for b in range(B_SH):
        mrow = mask[b]                                      # [2048]
        marker = np.where(mrow > 0, np.arange(S, dtype=np.float32), -1.0)
        markerT[:, 16 * b:16 * (b + 1)] = marker.reshape(128, 16)
    return np.ascontiguousarray(
        np.concatenate([id128, svalp1T, baseT, markerT], axis=1))  # [128,192]


def _run(inputs, trace=False, tmpdir=None):
    if trace:
        _ensure_ntff_hook()
    x = np.asarray(inputs["x"], dtype=np.float32)
    ids = np.asarray(inputs["input_ids"])
    stid = int(np.asarray(inputs["start_token_id"]))
    ctrl = np.asarray(inputs["control_emb"], dtype=np.float32)
    seq = np.asarray(inputs["sequence_emb"], dtype=np.float32)

    if "nc" not in _CACHE:
        _CACHE["nc"] = _build_bass()
    nc = _CACHE["nc"]

    table = np.concatenate(
        [ctrl, seq, np.zeros((1, D), dtype=np.float32)], axis=0)
    table_bf16 = table.astype(ml_dtypes.bfloat16)

    pos_ok = np.arange(S) >= N_CTRL
    mask = ((ids == stid) & pos_ok[None, :]).astype(np.float32)    # [B, S]

    in_maps = []
    for i in range(N_CORES):
        b0 = i * B_SH
        xsh = np.ascontiguousarray(x[b0:b0 + B_SH].reshape(TOK, D))
        cst = _consts(mask[b0:b0 + B_SH])
        in_maps.append({"x": xsh, "consts": cst, "table": table_bf16})

    res = run_bass_kernel_spmd(nc, in_maps, core_ids=list(range(N_CORES)),
                               trace=trace, tmpdir=tmpdir)
    out = np.concatenate(
        [np.asarray(res.results[i]["out"]).reshape(B_SH, S, D)
         for i in range(N_CORES)], axis=0)
    return out, res


def kernel(**inputs) -> np.ndarray:
    out, _ = _run(inputs, trace=bool(os.environ.get("BASS_TRACE")))
    return out


# revision 19
# speedup vs baseline: 1.1563x; 1.1563x over previous
"""AdaptiveLocalPositionEmbedding Trainium2 kernel (8 NeuronCores, data parallel).

out[b,s,:] = x[b,s,:] + pos_emb[b,s,:] where pos_emb is:
  - control_emb[s]            if s < 4 and no start-token segment covers s
  - sequence_emb[s - last]    if a start token (>= pos 4) precedes s and rel < 1003
  - 0                         otherwise
`last` = latest position <= s with input_ids == start_token_id (at pos >= 4).

Device work per core (2 batch rows): segment scan (cummax) over the start-token
mask, per-token index computation, indirect-DMA gather of 2KB table rows, add.
Host work: dtype casts, equality mask vs. the runtime scalar start_token_id,
table concat, shard/unshard.
"""

import os
import sys

import numpy as np

for _p in ("/opt/trn_rl_repo",):
    if _p not in sys.path:
        sys.path.insert(0, _p)

import ml_dtypes

from concourse import bacc, bass, mybir
from concourse.bass_utils import run_bass_kernel_spmd
from concourse.tile import TileContext

B, S, D = 16, 2048, 512
N_CORES = 8
B_SH = B // N_CORES            # 2 batch rows per core
TOK = B_SH * S                 # 4096 tokens per core
NT = TOK // 128                # 32 tiles of 128 tokens
NQ = 16                        # 128-token blocks per batch row (S / 128)
N_CTRL = 4
N_SEQ = 1003
ZERO_ROW = N_CTRL + N_SEQ      # 1007
TBL = ZERO_ROW + 1             # 1008 rows
F32 = mybir.dt.float32
BF16 = mybir.dt.bfloat16
I32 = mybir.dt.int32

_CACHE = {}


def _ensure_ntff_hook():
    """The agent image's antenv package lacks axon_hooks, so NTFF tracing
    silently degrades. Synthesize the module and register the boot script's
    ctypes-based profile hook so trace=True yields exec_time_ns."""
    if "antenv.axon_hooks" in sys.modules:
        return
    try:
        import types

        import antenv
        from trn_agent_boot.trn_boot import _ntff_profile_via_ctypes

        mod = types.ModuleType("antenv.axon_hooks")
        mod._hook = None

        def set_axon_ntff_profile_hook(h):
            mod._hook = h

        def get_axon_ntff_profile_hook():
            return mod._hook

        mod.set_axon_ntff_profile_hook = set_axon_ntff_profile_hook
        mod.get_axon_ntff_profile_hook = get_axon_ntff_profile_hook
        sys.modules["antenv.axon_hooks"] = mod
        antenv.axon_hooks = mod
        mod._hook = _ntff_profile_via_ctypes("/opt/axon/libaxon_pjrt.so")
    except Exception as e:  # tracing degrades; run still works
        print(f"NTFF hook registration failed: {e}", file=sys.stderr)


def _build_bass():
    nc = bacc.Bacc()
    x_h = nc.dram_tensor("x", [TOK, D], F32, kind="ExternalInput")
    # packed small inputs: [:, 0:128]=start mask, [:,128:256]=s+1,
    # [:,256:384]=base idx, [:,384:416]=32x32 identity, [:,416:417]=1.0
    cst_h = nc.dram_tensor("consts", [2 * NQ, 417], F32, kind="ExternalInput")
    table_h = nc.dram_tensor("table", [TBL, D], BF16, kind="ExternalInput")
    out_h = nc.dram_tensor("out", [TOK, D], F32, kind="ExternalOutput")

    QP = 2 * NQ  # 32 partitions used by the index pipeline

    with TileContext(nc) as tc:
        with (
            tc.tile_pool(name="const", bufs=1) as cpool,
            tc.tile_pool(name="work", bufs=3) as wpool,
            tc.tile_pool(name="psum", bufs=1, space="PSUM") as ppool,
        ):
            # ---- constants / small inputs (single DMA → single wait) ----
            cst = cpool.tile([QP, 417], F32)
            nc.gpsimd.dma_start(out=cst[:], in_=cst_h[:])
            sm = cst[:, 0:128]
            svalp1 = cst[:, 128:256]
            basei = cst[:, 256:384]
            # identities staged via DVE so matmuls wait on one semaphore only
            id32s = cpool.tile([QP, QP], F32)
            id1s = cpool.tile([1, 1], F32)
            nc.vector.tensor_copy(out=id32s[:], in_=cst[:, 384:416])
            nc.vector.tensor_copy(out=id1s[:], in_=cst[0:1, 416:417])
            id32t = id32s[:]
            id1t = id1s[:]

            # ---- marker = sm * (s+1) - 1  (s where start token, else -1) ----
            sA = cpool.tile([QP, 128], F32)
            sB = cpool.tile([QP, 128], F32)
            nc.vector.tensor_tensor(out=sA[:], in0=sm, in1=svalp1,
                                    op=mybir.AluOpType.mult)
            nc.vector.tensor_scalar_add(out=sA[:], in0=sA[:], scalar1=-1.0)

            # ---- inclusive cummax along free dim (within each 128 block) ----
            cur, nxt = sA, sB
            for k in (1, 2, 4, 8, 16, 32, 64):
                nc.vector.tensor_copy(out=nxt[:, :k], in_=cur[:, :k])
                nc.vector.tensor_tensor(out=nxt[:, k:], in0=cur[:, k:],
                                        in1=cur[:, : 128 - k],
                                        op=mybir.AluOpType.max)
                cur, nxt = nxt, cur
            # cur[q, p] = max over p' <= p of marker(q, p')

            # ---- cross-block exclusive cummax (per batch row) ----
            mbT_ps = ppool.tile([1, QP], F32, space="PSUM")
            nc.tensor.matmul(out=mbT_ps[:], lhsT=cur[:, 127:128], rhs=id32t,
                             start=True, stop=True)
            ex = cpool.tile([1, QP], F32)
            ex2 = cpool.tile([1, QP], F32)
            nc.vector.memset(ex[:], -1.0)
            # exclusive shift within each 16-block half
            nc.vector.tensor_copy(out=ex[:, 1:NQ], in_=mbT_ps[:, 0:NQ - 1])
            nc.vector.tensor_copy(out=ex[:, NQ + 1:QP], in_=mbT_ps[:, NQ:QP - 1])
            curX, nxtX = ex, ex2
            for k in (1, 2, 4, 8):
                for h in (0, NQ):
                    nc.vector.tensor_copy(out=nxtX[:, h:h + k],
                                          in_=curX[:, h:h + k])
                    nc.vector.tensor_tensor(out=nxtX[:, h + k:h + NQ],
                                            in0=curX[:, h + k:h + NQ],
                                            in1=curX[:, h:h + NQ - k],
                                            op=mybir.AluOpType.max)
                curX, nxtX = nxtX, curX
            pref_ps = ppool.tile([QP, 1], F32, space="PSUM")
            nc.tensor.matmul(out=pref_ps[:], lhsT=curX[:], rhs=id1t,
                             start=True, stop=True)
            pref = cpool.tile([QP, 1], F32)
            nc.vector.tensor_copy(out=pref[:], in_=pref_ps[:])

            # ---- last_start, rel, validity, final table index ----
            last = nxt  # reuse the other scan buffer
            nc.vector.tensor_tensor(out=last[:], in0=cur[:],
                                    in1=pref[:, 0:1].to_broadcast([QP, 128]),
                                    op=mybir.AluOpType.max)
            ge0 = cpool.tile([QP, 128], F32)
            nc.vector.tensor_scalar(out=ge0[:], in0=last[:], scalar1=0.0,
                                    scalar2=None, op0=mybir.AluOpType.is_ge)
            rel4 = cpool.tile([QP, 128], F32)
            # rel + 4 = (s + 1) + 3 - last
            nc.vector.tensor_tensor(out=rel4[:], in0=svalp1, in1=last[:],
                                    op=mybir.AluOpType.subtract)
            nc.vector.tensor_scalar_add(out=rel4[:], in0=rel4[:], scalar1=3.0)
            le = cpool.tile([QP, 128], F32)
            nc.vector.tensor_scalar(out=le[:], in0=rel4[:], scalar1=1006.0,
                                    scalar2=None, op0=mybir.AluOpType.is_le)
            valid = cpool.tile([QP, 128], F32)
            nc.vector.tensor_tensor(out=valid[:], in0=ge0[:], in1=le[:],
                                    op=mybir.AluOpType.mult)
            idxf = cpool.tile([QP, 128], F32)
            # idx = base + valid * (rel4 - base)
            nc.vector.tensor_tensor(out=idxf[:], in0=rel4[:], in1=basei,
                                    op=mybir.AluOpType.subtract)
            nc.vector.tensor_tensor(out=idxf[:], in0=idxf[:], in1=valid[:],
                                    op=mybir.AluOpType.mult)
            nc.vector.tensor_tensor(out=idxf[:], in0=idxf[:], in1=basei,
                                    op=mybir.AluOpType.add)

            # ---- transpose to gather layout: idxT[p, q] = idx(token q*128+p) ----
            idxT_ps = ppool.tile([128, QP], F32, space="PSUM")
            nc.tensor.matmul(out=idxT_ps[:], lhsT=idxf[:], rhs=id32t,
                             start=True, stop=True)
            idxT = cpool.tile([128, QP], I32)
            nc.vector.tensor_copy(out=idxT[:], in_=idxT_ps[:])

            # ---- main data path: 4 macro tiles of 1024 tokens ----
            # 1MiB batched x loads/stores; 8 per-128-row indirect gathers per
            # macro tile (bf16 rows) land in slices of one emb tile.
            MC = 8  # 128-token blocks per macro tile
            for j in range(NT // MC):
                xt = wpool.tile([128, MC, D], F32, tag="xt")
                emb = wpool.tile([128, MC, D], BF16, tag="emb")
                nc.sync.dma_start(
                    out=xt[:, :, :],
                    in_=x_h[j * MC * 128:(j + 1) * MC * 128, :].rearrange(
                        "(c p) d -> p c d", c=MC, p=128))
                for c in range(MC):
                    q = j * MC + c
                    nc.gpsimd.indirect_dma_start(
                        out=emb[:, c, :],
                        out_offset=None,
                        in_=table_h[:],
                        in_offset=bass.IndirectOffsetOnAxis(
                            ap=idxT[:, q:q + 1], axis=0),
                    )
                nc.vector.tensor_tensor(out=xt[:, :, :], in0=xt[:, :, :],
                                        in1=emb[:, :, :],
                                        op=mybir.AluOpType.add)
                nc.sync.dma_start(
                    out=out_h[j * MC * 128:(j + 1) * MC * 128, :].rearrange(
                        "(c p) d -> p c d", c=MC, p=128),
                    in_=xt[:, :, :])
    nc.compile()
    return nc


def _consts():
    s = np.arange(S, dtype=np.float32).reshape(NQ, 128)
    svalp1 = np.tile(s + 1.0, (2, 1)).astype(np.float32)           # [32,128]
    base = np.where(s < N_CTRL, s, float(ZERO_ROW))
    base = np.tile(base, (2, 1)).astype(np.float32)                # [32,128]
    return svalp1, base


def _run(inputs, trace=False, tmpdir=None):
    if trace:
        _ensure_ntff_hook()
    x = np.asarray(inputs["x"], dtype=np.float32)
    ids = np.asarray(inputs["input_ids"])
    stid = int(np.asarray(inputs["start_token_id"]))
    ctrl = np.asarray(inputs["control_emb"], dtype=np.float32)
    seq = np.asarray(inputs["sequence_emb"], dtype=np.float32)

    if "nc" not in _CACHE:
        _CACHE["nc"] = _build_bass()
    nc = _CACHE["nc"]

    table = np.concatenate(
        [ctrl, seq, np.zeros((1, D), dtype=np.float32)],
        axis=0).astype(ml_dtypes.bfloat16)
    svalp1, base = _consts()

    pos_ok = np.arange(S) >= N_CTRL
    mask = ((ids == stid) & pos_ok[None, :]).astype(np.float32)    # [B, S]

    in_maps = []
    for i in range(N_CORES):
        b0 = i * B_SH
        xsh = np.ascontiguousarray(
            x[b0:b0 + B_SH].reshape(TOK, D))
        msh = mask[b0:b0 + B_SH].reshape(2 * NQ, 128)
        id32 = np.eye(2 * NQ, 2 * NQ, dtype=np.float32)
        ones = np.ones((2 * NQ, 1), dtype=np.float32)
        cst = np.ascontiguousarray(
            np.concatenate([msh, svalp1, base, id32, ones], axis=1))  # [32, 417]
        in_maps.append({"x": xsh, "consts": cst, "table": table})

    res = run_bass_kernel_spmd(nc, in_maps, core_ids=list(range(N_CORES)),
                               trace=trace, tmpdir=tmpdir)
    out = np.concatenate(
        [res.results[i]["out"].reshape(B_SH, S, D) for i in range(N_CORES)],
        axis=0)
    return out, res


def kernel(**inputs) -> np.ndarray:
    out, _ = _run(inputs, trace=bool(os.environ.get("BASS_TRACE")))
    return out


# revision 21
# speedup vs baseline: 1.2978x; 1.1224x over previous
"""AdaptiveLocalPositionEmbedding Trainium2 kernel (8 NeuronCores, data parallel).

out[b,s,:] = x[b,s,:] + pos_emb[b,s,:] where pos_emb is
  control_emb[s] (s<4, before any start segment), sequence_emb[s-last] for the
  latest start token position last<=s (planted at pos>=4, rel<1003), else 0.

Per core (2 batch rows, 4096 tokens): the device computes the segment scan
(cummax over start-token markers) and per-token table indices, then fetches
table rows with dma_gather (4x1024 rows across 4 SWDGE queues, bf16 to halve
gather HBM traffic) and adds them into 1MiB-batched x tiles. Host does dtype
casts, the ==start_token_id compare (runtime scalar), layout packing, and
shard/unshard.

Token/slot layout: gather slot i <-> token t=i (t = b*2048 + s). dma_gather
takes idx i at partition i%16, col i//16 (replicated x8 across partition
groups for the 8 DGE cores) and writes row i to dst[i%128, i//128], matching
x tiles [128, c, 512]. The index pipeline runs in a transposed space:
markerT[mm, b*16+g] for s = mm*16+g; cummax over g on the free dim (in-place
log-shift max - max is idempotent so overlapping in-place steps are safe),
cross-column scan via PE transpose, final PE transpose emits
G[g, b*128+mm] = idx as int16.
"""

import os
import sys

import numpy as np

for _p in ("/opt/trn_rl_repo",):
    if _p not in sys.path:
        sys.path.insert(0, _p)

import ml_dtypes

from concourse import bacc, bass, library_config, mybir
from concourse.bass_utils import run_bass_kernel_spmd
from concourse.tile import TileContext

B, S, D = 16, 2048, 512
N_CORES = 8
B_SH = B // N_CORES            # 2 batch rows per core
TOK = B_SH * S                 # 4096 tokens per core
N_CTRL = 4
N_SEQ = 1003
ZERO_ROW = N_CTRL + N_SEQ      # 1007
TBL = ZERO_ROW + 1             # 1008 rows
NG = 4                         # gather calls (1024 rows each, one per queue)
GI = TOK // NG                 # 1024 idxs per gather
F32 = mybir.dt.float32
BF16 = mybir.dt.bfloat16
I16 = mybir.dt.int16

_CACHE = {}


def _ensure_ntff_hook():
    """The agent image's antenv package lacks axon_hooks, so NTFF tracing
    silently degrades. Synthesize the module and register the boot script's
    ctypes-based profile hook so trace=True yields exec_time_ns."""
    if "antenv.axon_hooks" in sys.modules:
        return
    try:
        import types

        import antenv
        from trn_agent_boot.trn_boot import _ntff_profile_via_ctypes

        mod = types.ModuleType("antenv.axon_hooks")
        mod._hook = None

        def set_axon_ntff_profile_hook(h):
            mod._hook = h

        def get_axon_ntff_profile_hook():
            return mod._hook

        mod.set_axon_ntff_profile_hook = set_axon_ntff_profile_hook
        mod.get_axon_ntff_profile_hook = get_axon_ntff_profile_hook
        sys.modules["antenv.axon_hooks"] = mod
        antenv.axon_hooks = mod
        mod._hook = _ntff_profile_via_ctypes("/opt/axon/libaxon_pjrt.so")
    except Exception as e:  # tracing degrades; run still works
        print(f"NTFF hook registration failed: {e}", file=sys.stderr)


def _build_bass():
    nc = bacc.Bacc("TRN2", num_swdge_queues=4)
    x_h = nc.dram_tensor("x", [TOK, D], F32, kind="ExternalInput")
    # consts [128, 192]: 0:128 id128, 128:144 sval4T (s+4), 144:160 baseT,
    # 160:192 markerT (per-core start-token markers, transposed layout)
    cst_h = nc.dram_tensor("consts", [128, 192], F32, kind="ExternalInput")
    table_h = nc.dram_tensor("table", [TBL, D], BF16, kind="ExternalInput")
    out_h = nc.dram_tensor("out", [TOK, D], F32, kind="ExternalOutput")

    with TileContext(nc) as tc:
        with (
            tc.tile_pool(name="const", bufs=1) as cpool,
            tc.tile_pool(name="work", bufs=3) as wpool,
            tc.tile_pool(name="psum", bufs=1, space="PSUM") as ppool,
        ):
            # pull the gpsimd DMA-gather ucode in during startup so the
            # gather stream is not stalled on the library reload DMA
            nc.gpsimd.load_library(library_config.mlp)
            cst = cpool.tile([128, 192], F32)
            nc.sync.dma_start(out=cst[:], in_=cst_h[:])
            id128 = cst[:, 0:128]
            id1 = cst[0:1, 0:1]
            sval4T = cst[:, 128:144]
            baseT = cst[:, 144:160]

            # G[g, b*128+mm] = table index for token t=b*2048+mm*16+g, int16,
            # replicated x8 across 16-partition groups for the DGE cores.
            G = cpool.tile([128, 2 * 128], I16)

            for b in range(B_SH):
                mk = cst[:, 160 + 16 * b:160 + 16 * (b + 1)]   # [128,16]
                # inclusive cummax along g (within each 16-token column)
                sA = cpool.tile([128, 16], F32, tag=f"sA{b}")
                nc.vector.tensor_copy(out=sA[:], in_=mk)
                for k in (1, 2, 4, 8):
                    nc.vector.tensor_tensor(out=sA[:, k:], in0=sA[:, k:],
                                            in1=sA[:, :16 - k],
                                            op=mybir.AluOpType.max)
                # cross-column exclusive cummax over mm (128 columns per b)
                cmT_ps = ppool.tile([1, 128], F32, space="PSUM", tag=f"cm{b}")
                nc.tensor.matmul(out=cmT_ps[:], lhsT=sA[:, 15:16], rhs=id128,
                                 start=True, stop=True)
                ex = cpool.tile([1, 128], F32, tag=f"ex{b}")
                nc.vector.memset(ex[:, 0:1], -1.0)
                nc.vector.tensor_copy(out=ex[:, 1:128], in_=cmT_ps[:, 0:127])
                for k in (1, 2, 4, 8, 16, 32, 64):
                    nc.vector.tensor_tensor(out=ex[:, k:], in0=ex[:, k:],
                                            in1=ex[:, :128 - k],
                                            op=mybir.AluOpType.max)
                pref_ps = ppool.tile([128, 1], F32, space="PSUM", tag=f"pf{b}")
                nc.tensor.matmul(out=pref_ps[:], lhsT=ex[:], rhs=id1,
                                 start=True, stop=True)
                pref = cpool.tile([128, 1], F32, tag=f"pref{b}")
                nc.vector.tensor_copy(out=pref[:], in_=pref_ps[:])

                # last_start, rel+4, validity, table index
                nc.vector.tensor_tensor(out=sA[:], in0=sA[:],
                                        in1=pref[:, 0:1].to_broadcast([128, 16]),
                                        op=mybir.AluOpType.max)
                ge0 = cpool.tile([128, 16], F32, tag=f"ge0{b}")
                nc.vector.tensor_scalar(out=ge0[:], in0=sA[:], scalar1=0.0,
                                        scalar2=None, op0=mybir.AluOpType.is_ge)
                rel4 = cpool.tile([128, 16], F32, tag=f"rel4{b}")
                nc.vector.tensor_tensor(out=rel4[:], in0=sval4T, in1=sA[:],
                                        op=mybir.AluOpType.subtract)
                le = cpool.tile([128, 16], F32, tag=f"le{b}")
                nc.vector.tensor_scalar(out=le[:], in0=rel4[:], scalar1=1006.0,
                                        scalar2=None, op0=mybir.AluOpType.is_le)
                valid = cpool.tile([128, 16], F32, tag=f"va{b}")
                nc.vector.tensor_tensor(out=valid[:], in0=ge0[:], in1=le[:],
                                        op=mybir.AluOpType.mult)
                idxf = cpool.tile([128, 16], F32, tag=f"ix{b}")
                nc.vector.tensor_tensor(out=idxf[:], in0=rel4[:], in1=baseT,
                                        op=mybir.AluOpType.subtract)
                nc.vector.tensor_tensor(out=idxf[:], in0=idxf[:], in1=valid[:],
                                        op=mybir.AluOpType.mult)
                nc.vector.tensor_tensor(out=idxf[:], in0=idxf[:], in1=baseT,
                                        op=mybir.AluOpType.add)

                gT_ps = ppool.tile([16, 128], F32, space="PSUM", tag=f"gt{b}")
                nc.tensor.matmul(out=gT_ps[:], lhsT=idxf[:], rhs=id128,
                                 start=True, stop=True)
                nc.vector.tensor_copy(out=G[0:16, 128 * b:128 * (b + 1)],
                                      in_=gT_ps[:])
                # replicate this half to the other 7 partition groups
                for r in range(1, 8):
                    nc.sync.dma_start(
                        out=G[16 * r:16 * (r + 1), 128 * b:128 * (b + 1)],
                        in_=G[0:16, 128 * b:128 * (b + 1)])

            # ---- main data path: 4 macro tiles of 1024 tokens ----
            for j in range(NG):
                xt = wpool.tile([128, GI // 128, D], F32, tag="xt")
                emb = wpool.tile([128, GI // 128, D], BF16, tag="emb")
                nc.sync.dma_start(
                    out=xt[:, :, :],
                    in_=x_h[GI * j:GI * (j + 1), :].rearrange(
                        "(c p) d -> p c d", c=GI // 128, p=128))
                nc.gpsimd.dma_gather(
                    emb[:, :, :], table_h[:],
                    G[:, (GI // 16) * j:(GI // 16) * (j + 1)],
                    GI, GI, D, queue_num=j)
                nc.vector.tensor_tensor(
                    out=xt[:, :, :], in0=xt[:, :, :], in1=emb[:, :, :],
                    op=mybir.AluOpType.add)
                nc.sync.dma_start(
                    out=out_h[GI * j:GI * (j + 1), :].rearrange(
                        "(c p) d -> p c d", c=GI // 128, p=128),
                    in_=xt[:, :, :])
    nc.compile()
    return nc


def _consts(mask):
    """mask: [2, 2048] float 0/1 start-token mask for this core's rows."""
    mm = np.arange(128, dtype=np.float32)[:, None]
    g = np.arange(16, dtype=np.float32)[None, :]
    sv = mm * 16 + g                                        # [128,16] s value
    sval4T = (sv + 4.0).astype(np.float32)
    baseT = np.where(sv < N_CTRL, sv, float(ZERO_ROW)).astype(np.float32)
    id128 = np.eye(128, dtype=np.float32)
    markerT = np.empty((128, 2 * 16), dtype=np.float32)
    for b in range(B_SH):
        marker = np.where(mask[b] > 0, np.arange(S, dtype=np.float32), -1.0)
        markerT[:, 16 * b:16 * (b + 1)] = marker.reshape(128, 16)
    return np.ascontiguousarray(
        np.concatenate([id128, sval4T, baseT, markerT], axis=1))  # [128,192]


def _run(inputs, trace=False, tmpdir=None):
    if trace:
        _ensure_ntff_hook()
    x = np.asarray(inputs["x"], dtype=np.float32)
    ids = np.asarray(inputs["input_ids"])
    stid = int(np.asarray(inputs["start_token_id"]))
    ctrl = np.asarray(inputs["control_emb"], dtype=np.float32)
    seq = np.asarray(inputs["sequence_emb"], dtype=np.float32)

    if "nc" not in _CACHE:
        _CACHE["nc"] = _build_bass()
    nc = _CACHE["nc"]

    table = np.concatenate(
        [ctrl, seq, np.zeros((1, D), dtype=np.float32)],
        axis=0).astype(ml_dtypes.bfloat16)

    pos_ok = np.arange(S) >= N_CTRL
    mask = ((ids == stid) & pos_ok[None, :]).astype(np.float32)    # [B, S]

    in_maps = []
    for i in range(N_CORES):
        b0 = i * B_SH
        xsh = np.ascontiguousarray(x[b0:b0 + B_SH].reshape(TOK, D))
        cst = _consts(mask[b0:b0 + B_SH])
        in_maps.append({"x": xsh, "consts": cst, "table": table})

    res = run_bass_kernel_spmd(nc, in_maps, core_ids=list(range(N_CORES)),
                               trace=trace, tmpdir=tmpdir)
    out = np.concatenate(
        [np.asarray(res.results[i]["out"]).reshape(B_SH, S, D)
         for i in range(N_CORES)], axis=0)
    return out, res


def kernel(**inputs) -> np.ndarray:
    out, _ = _run(inputs, trace=bool(os.environ.get("BASS_TRACE")))
    return out
